# revision 4
# baseline (speedup 1.0000x reference)
"""Trainium2 Bass kernel for nn_CelestialWaveAggregator.

Math: out[b,s,c] = tanh(h_c(agg[b,s,c])) where agg = wave_features @ M.T (M is
the per-body softmax aggregation matrix over ragged wave groups) and h_c is the
per-body 1->32->64->32->1 gelu MLP collapsed to a *univariate* function of the
aggregated scalar.

Device strategy (8 cores, batch-sharded 4096*4 rows/core):
  - PE: agg matmuls in fp32 (9 replica-masked weight matrices accumulate a
    [117, N] PSUM tile that holds 9 different row-chunks x 13 bodies on
    partitions), plus final [117,128]->[128,117] transposes back to row-major.
  - DVE: per-partition affine+clamp, then modified-Horner polynomial
    b <- (b + e_k) * t  via scalar_tensor_tensor (one op per degree) with
    per-partition (per-body) Chebyshev-fit coefficients.
  - ACT: final tanh with per-partition bias (absorbs the poly constant term),
    and PSUM->SBUF copies of the output transposes.
The polynomial fit of tanh(h_c(x)) is computed on host from the (tiny) MLP
weights passed to the kernel; absmax error of the fit is ~3e-5.
"""

import math
import os

import numpy as np

# ---- problem constants (hardcoded per contract) ----
LENS = np.array([9, 9, 9, 9, 9, 9, 9, 9, 9, 9, 12, 8, 3])
STARTS = np.concatenate([[5], 5 + np.cumsum(LENS)[:-1]])
MAXW, NW, NB = 12, 118, 13
B, S = 32, 4096
NCORES = 8
RPC = (B * S) // NCORES          # 16384 rows per core
NREP = 9                         # replica groups on partitions (9*13=117)
NP_USED = NREP * NB              # 117 used partitions
F = 1824                         # t-tile free length per partition (9*1824 >= 16384)
RPAD = NREP * F                  # 16416: xt padded with 32 junk columns
NCHUNK = 4
FW = F // NCHUNK                 # 456 columns per pipeline chunk
DEG = 20                         # polynomial degree
NSLC = 15                        # output transpose slices: 14*128 + 32 = 1824

_f64 = np.float64


def _erf(x):
    try:
        from scipy.special import erf
        return erf(x)
    except Exception:
        return np.vectorize(math.erf)(x)


def _gelu(x):
    return 0.5 * x * (1.0 + _erf(x / np.sqrt(2.0)))


def _build_M(agg_logits):
    """Dense [13, 118] aggregation matrix from ragged softmax groups."""
    al = np.asarray(agg_logits, _f64)
    valid = np.arange(MAXW)[None, :] < LENS[:, None]
    logits = np.where(valid, al, -np.inf)
    w = np.exp(logits - logits.max(axis=-1, keepdims=True))
    w = w / w.sum(axis=-1, keepdims=True)
    w = np.where(valid, w, 0.0)
    M = np.zeros((NB, NW))
    idx = np.clip(STARTS[:, None] + np.arange(MAXW)[None, :], 0, NW - 1)
    for c in range(NB):
        for j in range(MAXW):
            M[c, idx[c, j]] += w[c, j]
    return M


def _h_fn(x, c, W1, b1, W2, b2, W3, b3, W4, b4):
    """Pre-tanh univariate MLP for body c, float64."""
    a = x[:, None] * W1[c, 0][None, :] + b1[c]
    h1 = _gelu(a)
    h2 = _gelu(h1 @ W2[c] + b2[c])
    h3 = _gelu(h2 @ W3[c] + b3[c])
    return h3 @ W4[c][:, 0] + b4[c, 0]


def _fit_tables(inputs):
    """Host precompute: aggregation matrix, per-body poly fits, device consts."""
    M = _build_M(inputs["agg_logits"])
    W = {k: np.asarray(inputs[k], _f64) for k in
         ("W1", "b1", "W2", "b2", "W3", "b3", "W4", "b4")}

    # calibration: per-body agg range from the actual data (+ margin, clamped on device)
    X = np.asarray(inputs["wave_features"], np.float32).reshape(-1, NW)
    agg = X.astype(_f64) @ M.T
    lo = agg.min(axis=0)
    hi = agg.max(axis=0)
    m = 0.12 * (hi - lo)
    lo, hi = lo - m, hi + m
    mid = 0.5 * (lo + hi)
    invhalf = 2.0 / (hi - lo)

    # per-body weighted Chebyshev fit of h_c, evaluated through tanh
    coeffs = np.zeros((NB, DEG + 1))
    for c in range(NB):
        xs = np.linspace(lo[c], hi[c], 3001)
        hs = _h_fn(xs, c, **W)
        ys = np.tanh(hs)
        t = (xs - mid[c]) * invhalf[c]
        V = np.polynomial.chebyshev.chebvander(t, DEG)
        wgt = 1.0 / np.cosh(hs) ** 2 + 1e-4
        for _ in range(10):
            sw = np.sqrt(wgt)
            coef, *_r = np.linalg.lstsq(V * sw[:, None], hs * sw, rcond=None)
            err = np.abs(np.tanh(V @ coef) - ys)
            wgt = wgt * (1.0 + 1.5 * err / (err.max() + 1e-12))
        coeffs[c] = np.polynomial.chebyshev.cheb2poly(coef)

    # device constant tensors
    # Wm: [118, 9*117]; replica r's lhsT block has column (r*13+c) = M[c,:]*invhalf[c]
    Wm = np.zeros((NW, NREP * NP_USED), np.float32)
    Ms = (M * invhalf[:, None]).T  # [118, 13]
    for r in range(NREP):
        for c in range(NB):
            Wm[:, r * NP_USED + r * NB + c] = Ms[:, c]
    # consts: [117, DEG+2]: col0 = mid*invhalf; col 1+i = a_{DEG-i} (i=0..DEG-1); col DEG+1 = a_0
    consts = np.zeros((NP_USED, DEG + 2), np.float32)
    for r in range(NREP):
        for c in range(NB):
            q = r * NB + c
            consts[q, 0] = mid[c] * invhalf[c]
            for i in range(DEG):
                consts[q, 1 + i] = coeffs[c, DEG - i]
            consts[q, DEG + 1] = coeffs[c, 0]
    ident = np.eye(NP_USED, dtype=np.float32)
    return Wm, consts, ident


_PROGRAM = None


def _build_program():
    """Build + compile the (SPMD, per-core) Bass/Tile program once."""
    global _PROGRAM
    if _PROGRAM is not None:
        return _PROGRAM

    from contextlib import ExitStack
    import concourse.bass as bass
    import concourse.bacc as bacc
    import concourse.tile as tile
    import concourse.mybir as mybir
    from concourse._compat import axon_active

    f32 = mybir.dt.float32
    Alu = mybir.AluOpType
    Act = mybir.ActivationFunctionType

    nc = bacc.Bacc(
        "TRN2",
        target_bir_lowering=False,
        debug=not axon_active(),
        enable_asserts=True,
        num_devices=NCORES,
    )
    xt = nc.dram_tensor("xt", [NW, RPAD], f32, kind="ExternalInput").ap()
    wm = nc.dram_tensor("wm", [NW, NREP * NP_USED], f32, kind="ExternalInput").ap()
    cst = nc.dram_tensor("cst", [NP_USED, DEG + 2], f32, kind="ExternalInput").ap()
    idt = nc.dram_tensor("idt", [NP_USED, NP_USED], f32, kind="ExternalInput").ap()
    out = nc.dram_tensor("out", [RPC, NB], f32, kind="ExternalOutput").ap()

    with tile.TileContext(nc) as tc, ExitStack() as ctx:
        cpool = ctx.enter_context(tc.tile_pool(name="consts", bufs=1))
        xpool = ctx.enter_context(tc.tile_pool(name="xin", bufs=2 * NREP))
        ppool = ctx.enter_context(tc.tile_pool(name="ps", bufs=2, space="PSUM"))
        tpool = ctx.enter_context(tc.tile_pool(name="tt", bufs=2))
        bpool = ctx.enter_context(tc.tile_pool(name="bb", bufs=2))
        ypool = ctx.enter_context(tc.tile_pool(name="yy", bufs=1))
        opool = ctx.enter_context(tc.tile_pool(name="oo", bufs=1))
        qpool = ctx.enter_context(tc.tile_pool(name="pt", bufs=4, space="PSUM"))

        wm_sb = cpool.tile([NW, NREP * NP_USED], f32)
        nc.sync.dma_start(wm_sb[:], wm[:])
        cst_sb = cpool.tile([NP_USED, DEG + 2], f32)
        nc.sync.dma_start(cst_sb[:], cst[:])
        idt_sb = cpool.tile([NP_USED, NP_USED], f32)
        nc.sync.dma_start(idt_sb[:], idt[:])

        y_sb = ypool.tile([NP_USED, F], f32)
        out_sb = opool.tile([128, NSLC * NP_USED], f32)

        mid_ap = cst_sb[:, 0:1]
        a0_ap = cst_sb[:, DEG + 1:DEG + 2]

        for j in range(NCHUNK):
            xts = []
            for r in range(NREP):
                xt_t = xpool.tile([NW, FW], f32, tag="xin")
                nc.sync.dma_start(xt_t[:], xt[:, r * F + j * FW: r * F + (j + 1) * FW])
                xts.append(xt_t)
            ps = ppool.tile([NP_USED, FW], f32)
            for r in range(NREP):
                nc.tensor.matmul(
                    ps[:],
                    wm_sb[:, r * NP_USED:(r + 1) * NP_USED],
                    xts[r][:],
                    start=(r == 0),
                    stop=(r == NREP - 1),
                )
            # t = clamp(agg*invhalf - mid', -1, 1)
            t_t = tpool.tile([NP_USED, FW], f32)
            nc.vector.tensor_scalar(t_t[:], ps[:], mid_ap, 1.0,
                                    op0=Alu.subtract, op1=Alu.min)
            nc.vector.tensor_scalar_max(t_t[:], t_t[:], -1.0)
            # modified Horner: b = a_D*t; b = (b + a_k)*t for k=D-1..1
            b_t = bpool.tile([NP_USED, FW], f32)
            nc.vector.tensor_scalar_mul(b_t[:], t_t[:], cst_sb[:, 1:2])
            for i in range(1, DEG):
                nc.vector.scalar_tensor_tensor(
                    b_t[:], b_t[:], cst_sb[:, 1 + i:2 + i], t_t[:],
                    op0=Alu.add, op1=Alu.mult,
                )
            # y = tanh(b + a_0)
            nc.scalar.activation(y_sb[:, j * FW:(j + 1) * FW], b_t[:],
                                 Act.Tanh, bias=a0_ap)

        # transpose [117, 128]-slices -> [128, 117] and stage to SBUF
        for s in range(NSLC):
            w = 128 if s < NSLC - 1 else F - 128 * (NSLC - 1)
            pt = qpool.tile([128, NP_USED], f32)
            nc.tensor.transpose(pt[0:w, :], y_sb[:, s * 128: s * 128 + w], idt_sb[:])
            nc.scalar.copy(out_sb[0:w, s * NP_USED:(s + 1) * NP_USED], pt[0:w, :])

        # store: rows r*F + s*128 + p, cols c  <-  out_sb[p, s*117 + r*13 + c]
        osr = out_sb[:].rearrange("p (s q) -> p s q", q=NP_USED)
        for r in range(NREP):
            dram = out[r * F: r * F + 14 * 128, :].rearrange("(s p) c -> p s c", p=128)
            nc.sync.dma_start(dram, osr[:, 0:14, r * NB:(r + 1) * NB])
            if r < NREP - 1:
                nc.sync.dma_start(
                    out[r * F + 1792: r * F + 1824, :],
                    out_sb[0:32, 14 * NP_USED + r * NB: 14 * NP_USED + (r + 1) * NB],
                )

    nc.compile()
    _PROGRAM = nc
    return nc


LAST_EXEC_NS = None


def kernel(**inputs) -> np.ndarray:
    global LAST_EXEC_NS
    from concourse.bass_utils import run_bass_kernel_spmd

    Wm, consts, ident = _fit_tables(inputs)
    X = np.ascontiguousarray(
        np.asarray(inputs["wave_features"], np.float32).reshape(B * S, NW)
    )

    in_maps = []
    for k in range(NCORES):
        xt_k = np.zeros((NW, RPAD), np.float32)
        xt_k[:, :RPC] = X[k * RPC:(k + 1) * RPC].T  # [118, 16384] + 32 pad cols
        in_maps.append({"xt": xt_k, "wm": Wm, "cst": consts, "idt": ident})

    nc = _build_program()
    trace = os.environ.get("BASS_KERNEL_PROFILE") == "1"
    res = run_bass_kernel_spmd(nc, in_maps, core_ids=list(range(NCORES)),
                               trace=trace)
    LAST_EXEC_NS = res.exec_time_ns
    outs = [np.asarray(res.results[k]["out"], np.float32) for k in range(NCORES)]
    return np.concatenate(outs, axis=0).reshape(B, S, NB)


# revision 8
# speedup vs baseline: 1.7893x; 1.7893x over previous
"""Trainium2 Bass kernel for nn_CelestialWaveAggregator.

Math: out[b,s,c] = tanh(h_c(agg[b,s,c])) where agg = wave_features @ M.T (M is
the per-body softmax aggregation matrix over ragged wave groups) and h_c is the
per-body 1->32->64->32->1 gelu MLP collapsed to a *univariate* function of the
aggregated scalar.

Device strategy (8 cores, batch-sharded 4096*4 rows/core):
  - PE: agg matmuls in fp32 (9 replica-masked weight matrices accumulate a
    [117, N] PSUM tile that holds 9 different row-chunks x 13 bodies on
    partitions), plus final [117,128]->[128,117] transposes back to row-major.
  - DVE: per-partition affine+clamp, then modified-Horner polynomial
    b <- (b + e_k) * t  via scalar_tensor_tensor (one op per degree) with
    per-partition (per-body) Chebyshev-fit coefficients.
  - ACT: final tanh with per-partition bias (absorbs the poly constant term),
    and PSUM->SBUF copies of the output transposes.
The polynomial fit of tanh(h_c(x)) is computed on host from the (tiny) MLP
weights passed to the kernel; absmax error of the fit is ~3e-5.
"""

import math
import os

import numpy as np

# ---- problem constants (hardcoded per contract) ----
LENS = np.array([9, 9, 9, 9, 9, 9, 9, 9, 9, 9, 12, 8, 3])
STARTS = np.concatenate([[5], 5 + np.cumsum(LENS)[:-1]])
MAXW, NW, NB = 12, 118, 13
B, S = 32, 4096
NCORES = 8
RPC = (B * S) // NCORES          # 16384 rows per core
NREP = 9                         # replica groups on partitions (9*13=117)
NP_USED = NREP * NB              # 117 used partitions
F = 1824                         # t-tile free length per partition (9*1824 >= 16384)
RPAD = NREP * F                  # 16416: xt padded with 32 junk columns
NCHUNK = 4
FW = F // NCHUNK                 # 456 columns per pipeline chunk
DEG = 20                         # polynomial degree
NSLC = 15                        # output transpose slices: 14*128 + 32 = 1824
MM_F32R = True                   # aggregation matmul in float32r (TF32) at 4x rate

_f64 = np.float64


def _erf(x):
    try:
        from scipy.special import erf
        return erf(x)
    except Exception:
        return np.vectorize(math.erf)(x)


def _gelu(x):
    return 0.5 * x * (1.0 + _erf(x / np.sqrt(2.0)))


def _build_M(agg_logits):
    """Dense [13, 118] aggregation matrix from ragged softmax groups."""
    al = np.asarray(agg_logits, _f64)
    valid = np.arange(MAXW)[None, :] < LENS[:, None]
    logits = np.where(valid, al, -np.inf)
    w = np.exp(logits - logits.max(axis=-1, keepdims=True))
    w = w / w.sum(axis=-1, keepdims=True)
    w = np.where(valid, w, 0.0)
    M = np.zeros((NB, NW))
    idx = np.clip(STARTS[:, None] + np.arange(MAXW)[None, :], 0, NW - 1)
    for c in range(NB):
        for j in range(MAXW):
            M[c, idx[c, j]] += w[c, j]
    return M


def _h_fn(x, c, W1, b1, W2, b2, W3, b3, W4, b4):
    """Pre-tanh univariate MLP for body c, float64."""
    a = x[:, None] * W1[c, 0][None, :] + b1[c]
    h1 = _gelu(a)
    h2 = _gelu(h1 @ W2[c] + b2[c])
    h3 = _gelu(h2 @ W3[c] + b3[c])
    return h3 @ W4[c][:, 0] + b4[c, 0]


def _fit_tables(inputs):
    """Host precompute: aggregation matrix, per-body poly fits, device consts."""
    M = _build_M(inputs["agg_logits"])
    W = {k: np.asarray(inputs[k], _f64) for k in
         ("W1", "b1", "W2", "b2", "W3", "b3", "W4", "b4")}

    # calibration: per-body agg range from the actual data (+ margin, clamped on device)
    X = np.asarray(inputs["wave_features"], np.float32).reshape(-1, NW)
    agg = X.astype(_f64) @ M.T
    lo = agg.min(axis=0)
    hi = agg.max(axis=0)
    m = 0.12 * (hi - lo)
    lo, hi = lo - m, hi + m
    mid = 0.5 * (lo + hi)
    invhalf = 2.0 / (hi - lo)

    # per-body weighted Chebyshev fit of h_c, evaluated through tanh
    coeffs = np.zeros((NB, DEG + 1))
    for c in range(NB):
        xs = np.linspace(lo[c], hi[c], 3001)
        hs = _h_fn(xs, c, **W)
        ys = np.tanh(hs)
        t = (xs - mid[c]) * invhalf[c]
        V = np.polynomial.chebyshev.chebvander(t, DEG)
        wgt = 1.0 / np.cosh(hs) ** 2 + 1e-4
        for _ in range(10):
            sw = np.sqrt(wgt)
            coef, *_r = np.linalg.lstsq(V * sw[:, None], hs * sw, rcond=None)
            err = np.abs(np.tanh(V @ coef) - ys)
            wgt = wgt * (1.0 + 1.5 * err / (err.max() + 1e-12))
        coeffs[c] = np.polynomial.chebyshev.cheb2poly(coef)

    # device constant tensors
    # Wm: [118, 9*117]; replica r's lhsT block has column (r*13+c) = M[c,:]*invhalf[c]
    Wm = np.zeros((NW, NREP * NP_USED), np.float32)
    Ms = (M * invhalf[:, None]).T  # [118, 13]
    for r in range(NREP):
        for c in range(NB):
            Wm[:, r * NP_USED + r * NB + c] = Ms[:, c]
    # consts: [117, DEG+2]: col0 = mid*invhalf; col 1+i = a_{DEG-i} (i=0..DEG-1); col DEG+1 = a_0
    consts = np.zeros((NP_USED, DEG + 2), np.float32)
    for r in range(NREP):
        for c in range(NB):
            q = r * NB + c
            consts[q, 0] = mid[c] * invhalf[c]
            for i in range(DEG):
                consts[q, 1 + i] = coeffs[c, DEG - i]
            consts[q, DEG + 1] = coeffs[c, 0]
    ident = np.eye(NP_USED, dtype=np.float32)
    return Wm, consts, ident


_PROGRAM = None


def _build_program():
    """Build + compile the (SPMD, per-core) Bass/Tile program once."""
    global _PROGRAM
    if _PROGRAM is not None:
        return _PROGRAM

    from contextlib import ExitStack
    import concourse.bass as bass
    import concourse.bacc as bacc
    import concourse.tile as tile
    import concourse.mybir as mybir
    from concourse._compat import axon_active

    f32 = mybir.dt.float32
    Alu = mybir.AluOpType
    Act = mybir.ActivationFunctionType

    nc = bacc.Bacc(
        "TRN2",
        target_bir_lowering=False,
        debug=not axon_active(),
        enable_asserts=True,
        num_devices=NCORES,
    )
    fmm = mybir.dt.float32r if MM_F32R else f32
    xt = nc.dram_tensor("xt", [NW, RPAD], fmm, kind="ExternalInput").ap()
    wm = nc.dram_tensor("wm", [NW, NREP * NP_USED], fmm, kind="ExternalInput").ap()
    cst = nc.dram_tensor("cst", [NP_USED, DEG + 2], f32, kind="ExternalInput").ap()
    idt = nc.dram_tensor("idt", [NP_USED, NP_USED], f32, kind="ExternalInput").ap()
    out = nc.dram_tensor("out", [RPC, NB], f32, kind="ExternalOutput").ap()

    with tile.TileContext(nc) as tc, ExitStack() as ctx:
        cpool = ctx.enter_context(tc.tile_pool(name="consts", bufs=1))
        xpool = ctx.enter_context(tc.tile_pool(name="xin", bufs=2 * NREP))
        ppool = ctx.enter_context(tc.tile_pool(name="ps", bufs=2, space="PSUM"))
        tpool = ctx.enter_context(tc.tile_pool(name="tt", bufs=2))
        bpool = ctx.enter_context(tc.tile_pool(name="bb", bufs=2))
        ypool = ctx.enter_context(tc.tile_pool(name="yy", bufs=1))
        opool = ctx.enter_context(tc.tile_pool(name="oo", bufs=1))
        qpool = ctx.enter_context(tc.tile_pool(name="pt", bufs=4, space="PSUM"))

        wm_sb = cpool.tile([NW, NREP * NP_USED], fmm)
        nc.gpsimd.dma_start(wm_sb[:], wm[:])
        cst_sb = cpool.tile([NP_USED, DEG + 2], f32)
        nc.gpsimd.dma_start(cst_sb[:], cst[:])
        idt_sb = cpool.tile([NP_USED, NP_USED], f32)
        nc.gpsimd.dma_start(idt_sb[:], idt[:])

        y_sb = ypool.tile([NP_USED, F], f32)
        out_sb = opool.tile([128, NSLC * NP_USED], f32)

        mid_ap = cst_sb[:, 0:1]
        a0_ap = cst_sb[:, DEG + 1:DEG + 2]

        for j in range(NCHUNK):
            xts = []
            for r in range(NREP):
                xt_t = xpool.tile([NW, FW], fmm, tag="xin")
                nc.gpsimd.dma_start(xt_t[:], xt[:, r * F + j * FW: r * F + (j + 1) * FW])
                xts.append(xt_t)
            ps = ppool.tile([NP_USED, FW], f32)
            for r in range(NREP):
                nc.tensor.matmul(
                    ps[:],
                    wm_sb[:, r * NP_USED:(r + 1) * NP_USED],
                    xts[r][:],
                    start=(r == 0),
                    stop=(r == NREP - 1),
                )
            # t = clamp(agg*invhalf - mid', -1, 1)
            t_t = tpool.tile([NP_USED, FW], f32)
            nc.vector.tensor_scalar(t_t[:], ps[:], mid_ap, 1.0,
                                    op0=Alu.subtract, op1=Alu.min)
            nc.vector.tensor_scalar_max(t_t[:], t_t[:], -1.0)
            # modified Horner: b = a_D*t; b = (b + a_k)*t for k=D-1..1
            b_t = bpool.tile([NP_USED, FW], f32)
            nc.vector.tensor_scalar_mul(b_t[:], t_t[:], cst_sb[:, 1:2])
            for i in range(1, DEG):
                nc.vector.scalar_tensor_tensor(
                    b_t[:], b_t[:], cst_sb[:, 1 + i:2 + i], t_t[:],
                    op0=Alu.add, op1=Alu.mult,
                )
            # y = tanh(b + a_0)
            nc.scalar.activation(y_sb[:, j * FW:(j + 1) * FW], b_t[:],
                                 Act.Tanh, bias=a0_ap)

        # transpose [117, 128]-slices -> [128, 117] and stage to SBUF
        for s in range(NSLC):
            w = 128 if s < NSLC - 1 else F - 128 * (NSLC - 1)
            pt = qpool.tile([128, NP_USED], f32)
            nc.tensor.transpose(pt[0:w, :], y_sb[:, s * 128: s * 128 + w], idt_sb[:])
            nc.scalar.copy(out_sb[0:w, s * NP_USED:(s + 1) * NP_USED], pt[0:w, :])

        # store: rows r*F + s*128 + p, cols c  <-  out_sb[p, s*117 + r*13 + c]
        osr = out_sb[:].rearrange("p (s q) -> p s q", q=NP_USED)
        for r in range(NREP):
            dram = out[r * F: r * F + 14 * 128, :].rearrange("(s p) c -> p s c", p=128)
            nc.sync.dma_start(dram, osr[:, 0:14, r * NB:(r + 1) * NB])
            if r < NREP - 1:
                nc.sync.dma_start(
                    out[r * F + 1792: r * F + 1824, :],
                    out_sb[0:32, 14 * NP_USED + r * NB: 14 * NP_USED + (r + 1) * NB],
                )

    nc.compile()
    _PROGRAM = nc
    return nc


LAST_EXEC_NS = None


def kernel(**inputs) -> np.ndarray:
    global LAST_EXEC_NS
    from concourse.bass_utils import run_bass_kernel_spmd

    Wm, consts, ident = _fit_tables(inputs)
    X = np.ascontiguousarray(
        np.asarray(inputs["wave_features"], np.float32).reshape(B * S, NW)
    )

    in_maps = []
    for k in range(NCORES):
        xt_k = np.zeros((NW, RPAD), np.float32)
        xt_k[:, :RPC] = X[k * RPC:(k + 1) * RPC].T  # [118, 16384] + 32 pad cols
        in_maps.append({"xt": xt_k, "wm": Wm, "cst": consts, "idt": ident})

    nc = _build_program()
    trace = os.environ.get("BASS_KERNEL_PROFILE") == "1"
    res = run_bass_kernel_spmd(nc, in_maps, core_ids=list(range(NCORES)),
                               trace=trace)
    LAST_EXEC_NS = res.exec_time_ns
    outs = [np.asarray(res.results[k]["out"], np.float32) for k in range(NCORES)]
    return np.concatenate(outs, axis=0).reshape(B, S, NB)


# revision 9
# speedup vs baseline: 1.8193x; 1.0168x over previous
"""Trainium2 Bass kernel for nn_CelestialWaveAggregator.

Math: out[b,s,c] = tanh(h_c(agg[b,s,c])) where agg = wave_features @ M.T (M is
the per-body softmax aggregation matrix over ragged wave groups) and h_c is the
per-body 1->32->64->32->1 gelu MLP collapsed to a *univariate* function of the
aggregated scalar.

Device strategy (8 cores, batch-sharded 2048*8 rows/core):
  - PE: agg matmuls in float32r (9... 8 replica-masked weight matrices
    accumulate a [104, N] PSUM tile holding 8 row-chunks x 13 bodies on
    partitions), plus final [104,128]->[128,104] transposes back to row-major.
  - ACT: PSUM->SBUF affine copy, final tanh with per-partition bias (absorbs
    the poly constant term), output-transpose PSUM->SBUF copies.
  - DVE: clamp, then modified-Horner polynomial  b <- (b + e_k) * t  via
    scalar_tensor_tensor (one op per degree) with per-partition (per-body)
    Chebyshev-fit coefficients.
The polynomial fit of tanh(h_c(x)) is computed on host from the (tiny) MLP
weights passed in; fit absmax error ~5e-5, f32r aggregation adds ~2e-4.
"""

import math
import os

import numpy as np

# ---- problem constants (hardcoded per contract) ----
LENS = np.array([9, 9, 9, 9, 9, 9, 9, 9, 9, 9, 12, 8, 3])
STARTS = np.concatenate([[5], 5 + np.cumsum(LENS)[:-1]])
MAXW, NW, NB = 12, 118, 13
B, S = 32, 4096
NCORES = 8
RPC = (B * S) // NCORES          # 16384 rows per core
NREP = 8                         # replica groups on partitions (8*13=104)
NP_USED = NREP * NB              # 104 used partitions
F = RPC // NREP                  # 2048 free columns per partition (exact)
NCHUNK = 2
FW = F // NCHUNK                 # 1024 columns per pipeline chunk
NBLK = F // 512                  # 4 output blocks of 512 cols (4 slices each)
DEG = 18                         # polynomial degree
MM_F32R = True                   # aggregation matmul in float32r (TF32 inputs)

_f64 = np.float64


def _erf(x):
    try:
        from scipy.special import erf
        return erf(x)
    except Exception:
        return np.vectorize(math.erf)(x)


def _gelu(x):
    return 0.5 * x * (1.0 + _erf(x / np.sqrt(2.0)))


def _build_M(agg_logits):
    """Dense [13, 118] aggregation matrix from ragged softmax groups."""
    al = np.asarray(agg_logits, _f64)
    valid = np.arange(MAXW)[None, :] < LENS[:, None]
    logits = np.where(valid, al, -np.inf)
    w = np.exp(logits - logits.max(axis=-1, keepdims=True))
    w = w / w.sum(axis=-1, keepdims=True)
    w = np.where(valid, w, 0.0)
    M = np.zeros((NB, NW))
    idx = np.clip(STARTS[:, None] + np.arange(MAXW)[None, :], 0, NW - 1)
    for c in range(NB):
        for j in range(MAXW):
            M[c, idx[c, j]] += w[c, j]
    return M


def _h_fn(x, c, W1, b1, W2, b2, W3, b3, W4, b4):
    """Pre-tanh univariate MLP for body c, float64."""
    a = x[:, None] * W1[c, 0][None, :] + b1[c]
    h1 = _gelu(a)
    h2 = _gelu(h1 @ W2[c] + b2[c])
    h3 = _gelu(h2 @ W3[c] + b3[c])
    return h3 @ W4[c][:, 0] + b4[c, 0]


def _fit_tables(inputs):
    """Host precompute: aggregation matrix, per-body poly fits, device consts."""
    M = _build_M(inputs["agg_logits"])
    W = {k: np.asarray(inputs[k], _f64) for k in
         ("W1", "b1", "W2", "b2", "W3", "b3", "W4", "b4")}

    # calibration: per-body agg range from the actual data (+ margin, clamped on device)
    X = np.asarray(inputs["wave_features"], np.float32).reshape(-1, NW)
    agg = X.astype(_f64) @ M.T
    lo = agg.min(axis=0)
    hi = agg.max(axis=0)
    m = 0.12 * (hi - lo)
    lo, hi = lo - m, hi + m
    mid = 0.5 * (lo + hi)
    invhalf = 2.0 / (hi - lo)

    # per-body weighted Chebyshev fit of h_c, evaluated through tanh
    coeffs = np.zeros((NB, DEG + 1))
    for c in range(NB):
        xs = np.linspace(lo[c], hi[c], 3001)
        hs = _h_fn(xs, c, **W)
        ys = np.tanh(hs)
        t = (xs - mid[c]) * invhalf[c]
        V = np.polynomial.chebyshev.chebvander(t, DEG)
        wgt = 1.0 / np.cosh(hs) ** 2 + 1e-4
        for _ in range(10):
            sw = np.sqrt(wgt)
            coef, *_r = np.linalg.lstsq(V * sw[:, None], hs * sw, rcond=None)
            err = np.abs(np.tanh(V @ coef) - ys)
            wgt = wgt * (1.0 + 1.5 * err / (err.max() + 1e-12))
        coeffs[c] = np.polynomial.chebyshev.cheb2poly(coef)

    # device constant tensors
    # Wm: [118, 8*104]; replica r's lhsT block has column (r*13+c) = M[c,:]*invhalf[c]
    Wm = np.zeros((NW, NREP * NP_USED), np.float32)
    Ms = (M * invhalf[:, None]).T  # [118, 13]
    for r in range(NREP):
        for c in range(NB):
            Wm[:, r * NP_USED + r * NB + c] = Ms[:, c]
    # consts: [104, DEG+2]: col0 = -mid*invhalf (ACT bias); col 1+i = a_{DEG-i};
    # col DEG+1 = a_0 (tanh bias)
    consts = np.zeros((NP_USED, DEG + 2), np.float32)
    for r in range(NREP):
        for c in range(NB):
            q = r * NB + c
            consts[q, 0] = -mid[c] * invhalf[c]
            for i in range(DEG):
                consts[q, 1 + i] = coeffs[c, DEG - i]
            consts[q, DEG + 1] = coeffs[c, 0]
    ident = np.eye(NP_USED, dtype=np.float32)
    return Wm, consts, ident


_PROGRAM = None


def _build_program():
    """Build + compile the (SPMD, per-core) Bass/Tile program once."""
    global _PROGRAM
    if _PROGRAM is not None:
        return _PROGRAM

    from contextlib import ExitStack
    import concourse.bacc as bacc
    import concourse.tile as tile
    import concourse.mybir as mybir
    from concourse._compat import axon_active

    f32 = mybir.dt.float32
    Alu = mybir.AluOpType
    Act = mybir.ActivationFunctionType

    nc = bacc.Bacc(
        "TRN2",
        target_bir_lowering=False,
        debug=not axon_active(),
        enable_asserts=True,
        num_devices=NCORES,
    )
    fmm = mybir.dt.float32r if MM_F32R else f32
    xt = nc.dram_tensor("xt", [NW, RPC], fmm, kind="ExternalInput").ap()
    wm = nc.dram_tensor("wm", [NW, NREP * NP_USED], fmm, kind="ExternalInput").ap()
    cst = nc.dram_tensor("cst", [NP_USED, DEG + 2], f32, kind="ExternalInput").ap()
    idt = nc.dram_tensor("idt", [NP_USED, NP_USED], f32, kind="ExternalInput").ap()
    out = nc.dram_tensor("out", [RPC, NB], f32, kind="ExternalOutput").ap()

    with tile.TileContext(nc) as tc, ExitStack() as ctx:
        cpool = ctx.enter_context(tc.tile_pool(name="consts", bufs=1))
        xpool = ctx.enter_context(tc.tile_pool(name="xin", bufs=2 * NREP))
        ppool = ctx.enter_context(tc.tile_pool(name="ps", bufs=2, space="PSUM"))
        tpool = ctx.enter_context(tc.tile_pool(name="tt", bufs=2))
        bpool = ctx.enter_context(tc.tile_pool(name="bb", bufs=2))
        ypool = ctx.enter_context(tc.tile_pool(name="yy", bufs=3))
        opool = ctx.enter_context(tc.tile_pool(name="oo", bufs=3))
        qpool = ctx.enter_context(tc.tile_pool(name="pt", bufs=4, space="PSUM"))

        wm_sb = cpool.tile([NW, NREP * NP_USED], fmm)
        nc.gpsimd.dma_start(wm_sb[:], wm[:])
        cst_sb = cpool.tile([NP_USED, DEG + 2], f32)
        nc.gpsimd.dma_start(cst_sb[:], cst[:])
        idt_sb = cpool.tile([NP_USED, NP_USED], f32)
        nc.gpsimd.dma_start(idt_sb[:], idt[:])

        nmid_ap = cst_sb[:, 0:1]
        a0_ap = cst_sb[:, DEG + 1:DEG + 2]

        for j in range(NCHUNK):
            xts = []
            for r in range(NREP):
                xt_t = xpool.tile([NW, FW], fmm, tag="xin")
                nc.gpsimd.dma_start(xt_t[:], xt[:, r * F + j * FW: r * F + (j + 1) * FW])
                xts.append(xt_t)
            ps = ppool.tile([NP_USED, FW], f32)
            for h in range(FW // 512):
                for r in range(NREP):
                    nc.tensor.matmul(
                        ps[:, h * 512:(h + 1) * 512],
                        wm_sb[:, r * NP_USED:(r + 1) * NP_USED],
                        xts[r][:, h * 512:(h + 1) * 512],
                        start=(r == 0),
                        stop=(r == NREP - 1),
                    )
            # t = clamp(agg*invhalf - mid', -1, 1): affine on ACT, clamp on DVE
            t_t = tpool.tile([NP_USED, FW], f32)
            nc.scalar.activation(t_t[:], ps[:], Act.Identity, bias=nmid_ap)
            nc.vector.tensor_scalar(t_t[:], t_t[:], 1.0, -1.0,
                                    op0=Alu.min, op1=Alu.max)
            # modified Horner: b = a_D*t; b = (b + a_k)*t for k=D-1..1
            b_t = bpool.tile([NP_USED, FW], f32)
            nc.vector.tensor_scalar_mul(b_t[:], t_t[:], cst_sb[:, 1:2])
            for i in range(1, DEG):
                nc.vector.scalar_tensor_tensor(
                    b_t[:], b_t[:], cst_sb[:, 1 + i:2 + i], t_t[:],
                    op0=Alu.add, op1=Alu.mult,
                )
            # per 512-block: tanh -> transpose -> stage -> store (overlaps next chunk)
            for hb in range(FW // 512):
                blk = j * (FW // 512) + hb
                y_t = ypool.tile([NP_USED, 512], f32, tag="yy")
                nc.scalar.activation(y_t[:], b_t[:, hb * 512:(hb + 1) * 512],
                                     Act.Tanh, bias=a0_ap)
                ob = opool.tile([128, 4 * NP_USED], f32, tag="oo")
                for s in range(4):
                    pt = qpool.tile([128, NP_USED], f32)
                    nc.tensor.transpose(pt[:], y_t[:, s * 128:(s + 1) * 128], idt_sb[:])
                    nc.scalar.copy(ob[:, s * NP_USED:(s + 1) * NP_USED], pt[:])
                # rows r*F + blk*512 + s*128 + p, cols c <- ob[p, s*104 + r*13 + c]
                obr = ob[:].rearrange("p (s q) -> p s q", q=NP_USED)
                for r in range(NREP):
                    dram = out[r * F + blk * 512: r * F + (blk + 1) * 512, :] \
                        .rearrange("(s p) c -> p s c", p=128)
                    nc.sync.dma_start(dram, obr[:, :, r * NB:(r + 1) * NB])

    nc.compile()
    _PROGRAM = nc
    return nc


LAST_EXEC_NS = None


def kernel(**inputs) -> np.ndarray:
    global LAST_EXEC_NS
    from concourse.bass_utils import run_bass_kernel_spmd

    Wm, consts, ident = _fit_tables(inputs)
    X = np.ascontiguousarray(
        np.asarray(inputs["wave_features"], np.float32).reshape(B * S, NW)
    )

    in_maps = []
    for k in range(NCORES):
        xt_k = np.ascontiguousarray(X[k * RPC:(k + 1) * RPC].T)  # [118, 16384]
        in_maps.append({"xt": xt_k, "wm": Wm, "cst": consts, "idt": ident})

    nc = _build_program()
    trace = os.environ.get("BASS_KERNEL_PROFILE") == "1"
    res = run_bass_kernel_spmd(nc, in_maps, core_ids=list(range(NCORES)),
                               trace=trace)
    LAST_EXEC_NS = res.exec_time_ns
    outs = [np.asarray(res.results[k]["out"], np.float32) for k in range(NCORES)]
    return np.concatenate(outs, axis=0).reshape(B, S, NB)
